# revision 8
# baseline (speedup 1.0000x reference)
"""Trainium2 Bass kernel for the MoE-routing Actor network (8 NeuronCores).

Sharding (per core i of 8):
  - Data-parallel gate/top-k + fc1 + LayerNorm1 + ReLU on the core's batch
    shard (512 rows), producing h1^T feature-major [8192, 512] bf16.
  - AllGather h1^T (batch-block concat) -> every core sees all 4096 rows.
  - Tensor-parallel fc2: core i holds fc2_W[:, i*1024:(i+1)*1024] bf16
    SBUF-resident; computes its 1024-feature slice for all batch in 8
    batch blocks of 512, LN2 stats AllReduced per block (pipelined).
  - LN2 + ReLU + expert-score multiply + group-of-16 mean via a constant
    selection matmul -> mixed^T slice [64, 512] per block, head partials,
    one AllReduce of head outputs [64, 4096], tanh/affine epilogue.
  - Output [64, 4096] (mean rows 0:32, log_std rows 32:64), host transposes.

All heavy matmuls run in bf16 (fp32 PE rate is 1/4 of bf16 on trn2).
"""

import numpy as np

import concourse.bass as bass
import concourse.bacc as bacc
import concourse.mybir as mybir
import concourse.tile as tile
from concourse.bass_utils import run_bass_kernel_spmd

F32 = mybir.dt.float32
BF16 = mybir.dt.bfloat16
AF = mybir.ActivationFunctionType
ALU = mybir.AluOpType
AX = mybir.AxisListType

N_CORES = 8
B, OBS, ACT_DIM, H, M, TOPK = 4096, 256, 32, 512, 16, 4
D = H * M          # 8192 trunk width
BL = B // N_CORES  # 512 local batch rows
DL = D // N_CORES  # 1024 local fc2 output features
HL = H // N_CORES  # 64 local mixed features
P = 128
NKT = D // P       # 64 k tiles over trunk width
NNT = DL // P      # 8 n tiles of local fc2 features
NBT = BL // P      # 4 batch tiles of the local shard
NRB = N_CORES      # 8 batch blocks of 512 in fc2 phase
LN_EPS = 1e-5
LOG_STD_MAX, LOG_STD_MIN = 2.0, -5.0
RG = [list(range(N_CORES))]

DEBUG_TAPS = False


def _consts():
    ident = np.eye(P, dtype=np.float32)
    ones_col = np.ones((P, 1), dtype=np.float32)
    ones_row = np.ones((1, P), dtype=np.float32)
    # E[k, p] = 1 if p % 16 == k  (broadcast 16 score rows over 128 partitions)
    E = np.zeros((M, P), dtype=np.float32)
    for p in range(P):
        E[p % M, p] = 1.0
    # S[n][p, g] = 1/16 if g == n*8 + p//16  (group-of-16 mean, n-th tile)
    S_all = np.zeros((NNT, P, HL), dtype=np.float32)
    for n in range(NNT):
        for p in range(P):
            S_all[n, p, n * (P // M) + p // M] = 1.0 / M
    return ident, ones_col, ones_row, E, S_all


def build_kernel():
    nc = bacc.Bacc(None, target_bir_lowering=False, num_devices=N_CORES)

    x_ext = nc.declare_dram_parameter("x", [BL, OBS], F32, isOutput=False)
    gw_ext = nc.declare_dram_parameter("gate_W", [OBS, M], F32, isOutput=False)
    gb_ext = nc.declare_dram_parameter("gate_b", [M], F32, isOutput=False)
    w1_ext = nc.declare_dram_parameter("fc1_W", [OBS, D], F32, isOutput=False)
    b1_ext = nc.declare_dram_parameter("fc1_b", [D], F32, isOutput=False)
    n1s_ext = nc.declare_dram_parameter("norm1_scale", [D], F32, isOutput=False)
    n1b_ext = nc.declare_dram_parameter("norm1_bias", [D], F32, isOutput=False)
    w2_ext = nc.declare_dram_parameter("fc2_W", [D, DL], F32, isOutput=False)
    b2_ext = nc.declare_dram_parameter("fc2_b", [DL], F32, isOutput=False)
    n2s_ext = nc.declare_dram_parameter("norm2_scale", [DL], F32, isOutput=False)
    n2b_ext = nc.declare_dram_parameter("norm2_bias", [DL], F32, isOutput=False)
    mw_ext = nc.declare_dram_parameter("mean_W", [HL, ACT_DIM], F32, isOutput=False)
    mb_ext = nc.declare_dram_parameter("mean_b", [ACT_DIM], F32, isOutput=False)
    lw_ext = nc.declare_dram_parameter("logstd_W", [HL, ACT_DIM], F32, isOutput=False)
    lb_ext = nc.declare_dram_parameter("logstd_b", [ACT_DIM], F32, isOutput=False)
    out_ext = nc.declare_dram_parameter("out", [2 * ACT_DIM, B], F32, isOutput=True)
    taps = {}
    if DEBUG_TAPS:
        taps["scores"] = nc.declare_dram_parameter("tap_scores", [M, BL], F32, isOutput=True)
        taps["h1"] = nc.declare_dram_parameter("tap_h1", [P, BL], F32, isOutput=True)
        taps["mixed"] = nc.declare_dram_parameter("tap_mixed", [HL, BL], F32, isOutput=True)

    ident_np, ones_col_np, ones_row_np, E_np, S_np = _consts()
    ident_dram = nc.inline_tensor(ident_np, name="ident")
    ones_col_dram = nc.inline_tensor(ones_col_np, name="ones_col")
    ones_row_dram = nc.inline_tensor(ones_row_np, name="ones_row")
    E_dram = nc.inline_tensor(E_np, name="Emat")
    S_flat = np.ascontiguousarray(S_np.transpose(1, 0, 2).reshape(P, NNT * HL))
    S_dram = nc.inline_tensor(S_flat, name="Smat")

    with tile.TileContext(nc) as tc:
        with (
            tc.tile_pool(name="cst", bufs=1) as cst,
            tc.tile_pool(name="dram", bufs=1, space="DRAM") as dram,
            tc.tile_pool(name="ppa", bufs=4, space="PSUM") as ppa,
            tc.tile_pool(name="pps", bufs=2, space="PSUM") as pps,
            tc.tile_pool(name="ppb", bufs=2, space="PSUM") as ppb,
        ):
            # ---------------- constants / small parameters ----------------
            ident = cst.tile([P, P], F32)
            nc.sync.dma_start(ident[:], ident_dram[:])
            ones_col_f = cst.tile([P, 1], F32)
            nc.sync.dma_start(ones_col_f[:], ones_col_dram[:])
            ones_col_b = cst.tile([P, 1], BF16)
            nc.vector.tensor_copy(ones_col_b[:], ones_col_f[:])
            ones_row_f = cst.tile([1, P], F32)
            nc.sync.dma_start(ones_row_f[:], ones_row_dram[:])
            ones_row_b = cst.tile([1, P], BF16)
            nc.vector.tensor_copy(ones_row_b[:], ones_row_f[:])
            Emat = cst.tile([M, P], F32)
            nc.sync.dma_start(Emat[:], E_dram[:])
            Smat = cst.tile([P, NNT * HL], BF16)

            fc1b = cst.tile([P, NKT], F32)
            nc.sync.dma_start(fc1b[:], b1_ext.ap().rearrange("(a b) -> b a", b=P))
            n1s = cst.tile([P, NKT], F32)
            nc.sync.dma_start(n1s[:], n1s_ext.ap().rearrange("(a b) -> b a", b=P))
            n1b = cst.tile([P, NKT], F32)
            nc.sync.dma_start(n1b[:], n1b_ext.ap().rearrange("(a b) -> b a", b=P))
            fc2b = cst.tile([P, NNT], F32)
            nc.sync.dma_start(fc2b[:], b2_ext.ap().rearrange("(a b) -> b a", b=P))
            n2s = cst.tile([P, NNT], F32)
            nc.sync.dma_start(n2s[:], n2s_ext.ap().rearrange("(a b) -> b a", b=P))
            n2b = cst.tile([P, NNT], F32)
            nc.sync.dma_start(n2b[:], n2b_ext.ap().rearrange("(a b) -> b a", b=P))
            gbf = cst.tile([1, M], F32)
            nc.sync.dma_start(gbf[:], gb_ext.ap().rearrange("(a b) -> a b", a=1))
            gbb = cst.tile([1, M], BF16)
            nc.vector.tensor_copy(gbb[:], gbf[:])
            mwt = cst.tile([HL, ACT_DIM], F32)
            nc.sync.dma_start(mwt[:], mw_ext[:])
            lwt = cst.tile([HL, ACT_DIM], F32)
            nc.sync.dma_start(lwt[:], lw_ext[:])
            mbt = cst.tile([ACT_DIM, 1], F32)
            nc.sync.dma_start(mbt[:], mb_ext.ap().rearrange("(a b) -> a b", b=1))
            lbt = cst.tile([ACT_DIM, 1], F32)
            nc.sync.dma_start(lbt[:], lb_ext.ap().rearrange("(a b) -> a b", b=1))
            gwb = cst.tile([P, 2 * M], BF16)
            for kt in range(2):
                gw_f = cst.tile([P, M], F32, tag="gw_f", bufs=2, name=f"gw_f{kt}")
                nc.sync.dma_start(gw_f[:], gw_ext[kt * P:(kt + 1) * P, :])
                nc.vector.tensor_copy(gwb[:, kt * M:(kt + 1) * M], gw_f[:])

            eps_t = cst.tile([1, 1], F32)
            nc.any.memset(eps_t[:], LN_EPS)
            xT = cst.tile([P, 2 * BL], BF16)  # x^T k-tiles side by side [k, b]
            scT = cst.tile([M, BL], F32)      # local top-k scores^T

            # DRAM bounce buffers
            ag_h1_in = dram.tile([D, BL], BF16)
            ag_h1_out = dram.tile([N_CORES * D, BL], BF16, addr_space="Shared")
            sc_in = dram.tile([M, BL], F32)
            sc_out = dram.tile([N_CORES * M, BL], F32, addr_space="Shared")
            st_in = [dram.tile([2, BL], F32, tag=f"st_in{r}", name=f"st_in{r}")
                     for r in range(NRB)]
            st_out = [dram.tile([2, BL], F32, addr_space="Shared",
                                tag=f"st_out{r}", name=f"st_out{r}")
                      for r in range(NRB)]
            hd_in = dram.tile([2 * ACT_DIM, B], F32)
            hd_out = dram.tile([2 * ACT_DIM, B], F32, addr_space="Shared")

            def ln_stats_to_bcast(pool, stx, stq, tagp, un):
                """[1,BL] f32 sum/sumsq vectors -> bf16 bcast tiles invB, nmuB."""
                def v(nm):
                    return pool.tile([1, BL], F32, tag=f"{tagp}v", bufs=6, name=f"{nm}{un}")
                mu = v("mu")
                nc.vector.tensor_scalar_mul(mu[:], stx[:], 1.0 / D)
                e2 = v("e2")
                nc.vector.tensor_scalar_mul(e2[:], stq[:], 1.0 / D)
                mu2 = v("mu2")
                nc.scalar.activation(mu2[:], mu[:], AF.Square)
                var = v("var")
                nc.vector.tensor_tensor(var[:], e2[:], mu2[:], op=ALU.subtract)
                sd = v("sd")
                nc.scalar.activation(sd[:], var[:], AF.Sqrt, bias=eps_t[:])
                inv = v("inv")
                nc.vector.reciprocal(inv[:], sd[:])
                nmuinv = v("nmuinv")
                nc.vector.tensor_tensor(nmuinv[:], mu[:], inv[:], op=ALU.mult)
                nmu2 = v("nmu2")
                nc.vector.tensor_scalar_mul(nmu2[:], nmuinv[:], -1.0)
                invB_ps = ppb.tile([P, BL], F32, tag="psb", name=f"invBps{un}")
                nc.tensor.matmul(invB_ps[:], ones_row_f[:], inv[:], start=True, stop=True)
                invB = pool.tile([P, BL], BF16, tag=f"{tagp}bc", bufs=4, name=f"invB{un}")
                nc.scalar.activation(invB[:], invB_ps[:], AF.Copy)
                nmuB_ps = ppb.tile([P, BL], F32, tag="psb", name=f"nmuBps{un}")
                nc.tensor.matmul(nmuB_ps[:], ones_row_f[:], nmu2[:], start=True, stop=True)
                nmuB = pool.tile([P, BL], BF16, tag=f"{tagp}bc", bufs=4, name=f"nmuB{un}")
                nc.scalar.activation(nmuB[:], nmuB_ps[:], AF.Copy)
                return invB, nmuB

            # ================= phase 0 + 1 (pool p1) =======================
            with tc.tile_pool(name="p1", bufs=1) as p1:
                Smat_f = p1.tile([P, NNT * HL], F32)
                nc.sync.dma_start(Smat_f[:], S_dram[:])
                nc.vector.tensor_copy(Smat[:], Smat_f[:])
                # ---- x^T ----
                for bt in range(NBT):
                    xl = p1.tile([P, OBS], F32, tag="xload", bufs=2, name=f"xl{bt}")
                    nc.sync.dma_start(xl[:], x_ext[bt * P:(bt + 1) * P, :])
                    for kt in range(2):
                        tp = ppb.tile([P, P], F32, tag="psb", name=f"xtp{bt}_{kt}")
                        nc.tensor.transpose(tp[:], xl[:, kt * P:(kt + 1) * P], ident[:])
                        nc.scalar.activation(
                            xT[:, kt * BL + bt * P: kt * BL + (bt + 1) * P],
                            tp[:], AF.Copy)

                # ---- gate + softmax + top-4 ----
                for bt in range(NBT):
                    gp = ppb.tile([P, M], F32, tag="psb", name=f"gp{bt}")
                    for kt in range(2):
                        nc.tensor.matmul(
                            gp[:], xT[:, kt * BL + bt * P: kt * BL + (bt + 1) * P],
                            gwb[:, kt * M:(kt + 1) * M], start=(kt == 0), stop=False)
                    nc.tensor.matmul(gp[:], ones_row_b[:], gbb[:], start=False, stop=True)

                    def g1(nm):
                        return p1.tile([P, 1], F32, tag="gs1", bufs=6, name=f"{nm}{bt}")

                    def g16(nm):
                        return p1.tile([P, M], F32, tag="gs16", bufs=6, name=f"{nm}{bt}")

                    gmax = g1("gmax")
                    nc.vector.tensor_reduce(gmax[:], gp[:], AX.X, ALU.max)
                    ngmax = g1("ngmax")
                    nc.vector.tensor_scalar_mul(ngmax[:], gmax[:], -1.0)
                    ge = g16("ge")
                    nc.scalar.activation(ge[:], gp[:], AF.Exp, bias=ngmax[:])
                    gsum = g1("gsum")
                    nc.vector.reduce_sum(gsum[:], ge[:], axis=AX.X)
                    grec = g1("grec")
                    nc.vector.reciprocal(grec[:], gsum[:])
                    s0 = g16("s0")
                    nc.vector.tensor_scalar_mul(s0[:], ge[:], grec[:])
                    mt4 = p1.tile([P, TOPK], F32, tag="gs4", bufs=2, name=f"mt4{bt}")
                    w = s0
                    for t in range(TOPK):
                        nc.vector.tensor_reduce(mt4[:, t:t + 1], w[:], AX.X, ALU.max)
                        if t < TOPK - 1:
                            msk = g16(f"msk{t}_")
                            nc.vector.tensor_scalar(msk[:], w[:], mt4[:, t:t + 1], None,
                                                    op0=ALU.is_ge)
                            w2_ = g16(f"w{t}_")
                            nc.vector.tensor_tensor(w2_[:], w[:], msk[:], op=ALU.subtract)
                            w = w2_
                    tsum = g1("tsum")
                    nc.vector.reduce_sum(tsum[:], mt4[:], axis=AX.X)
                    trec = g1("trec")
                    nc.vector.reciprocal(trec[:], tsum[:])
                    keep = g16("keep")
                    nc.vector.tensor_scalar(keep[:], s0[:], mt4[:, TOPK - 1:TOPK], None,
                                            op0=ALU.is_ge)
                    sn = g16("sn")
                    nc.vector.tensor_scalar_mul(sn[:], s0[:], trec[:])
                    sc = g16("sc")
                    nc.vector.tensor_tensor(sc[:], sn[:], keep[:], op=ALU.mult)
                    stp = ppb.tile([M, P], F32, tag="psb", name=f"stp{bt}")
                    nc.tensor.transpose(stp[:], sc[:], ident[:])
                    nc.scalar.activation(scT[:, bt * P:(bt + 1) * P], stp[:], AF.Copy)
                nc.sync.dma_start(sc_in[:], scT[:])
                nc.gpsimd.collective_compute(
                    "AllGather", ALU.bypass, replica_groups=RG,
                    ins=[sc_in.opt()], outs=[sc_out.opt()])
                if DEBUG_TAPS:
                    nc.sync.dma_start(taps["scores"][:], scT[:])

                # ---- fc1 (bf16) + LN1 stats ----
                w1b = []
                for kt in range(2):
                    w1t = p1.tile([P, D], BF16, tag=f"w1b{kt}", bufs=1, name=f"w1b{kt}")
                    for h in range(2):
                        w1f = p1.tile([P, D // 2], F32, tag="w1f", bufs=2,
                                      name=f"w1f{kt}_{h}")
                        nc.sync.dma_start(
                            w1f[:], w1_ext[kt * P:(kt + 1) * P,
                                           h * (D // 2):(h + 1) * (D // 2)])
                        nc.vector.tensor_copy(w1t[:, h * (D // 2):(h + 1) * (D // 2)],
                                              w1f[:])
                    w1b.append(w1t)

                h1raw = []
                st1x = pps.tile([1, BL], F32, tag="pss", name="st1x")
                st1q = pps.tile([1, BL], F32, tag="pss", name="st1q")
                for nt in range(NKT):
                    ps1 = ppa.tile([P, BL], F32, tag="psa", name=f"ps1_{nt}")
                    for kt in range(2):
                        nc.tensor.matmul(ps1[:], w1b[kt][:, nt * P:(nt + 1) * P],
                                         xT[:, kt * BL:(kt + 1) * BL],
                                         start=(kt == 0), stop=(kt == 1))
                    h1r = p1.tile([P, BL], BF16, tag=f"h1raw{nt}", bufs=1,
                                  name=f"h1r{nt}")
                    nc.scalar.activation(h1r[:], ps1[:], AF.Identity,
                                         bias=fc1b[:, nt:nt + 1])
                    h1raw.append(h1r)
                    sq = p1.tile([P, BL], BF16, tag="sq1", bufs=3, name=f"sq1_{nt}")
                    nc.scalar.activation(sq[:], h1r[:], AF.Square)
                    nc.tensor.matmul(st1x[:], ones_col_b[:], h1r[:],
                                     start=(nt == 0), stop=(nt == NKT - 1))
                    nc.tensor.matmul(st1q[:], ones_col_b[:], sq[:],
                                     start=(nt == 0), stop=(nt == NKT - 1))

                sx1 = p1.tile([1, BL], F32, tag="ln1v", bufs=6, name="sx1")
                nc.vector.tensor_copy(sx1[:], st1x[:])
                sq1v = p1.tile([1, BL], F32, tag="ln1v", bufs=6, name="sq1v")
                nc.vector.tensor_copy(sq1v[:], st1q[:])
                invB1, nmuB1 = ln_stats_to_bcast(p1, sx1, sq1v, "ln1", "L1")

                # ---- LN1 normalize + relu -> AG input ----
                for nt in range(NKT):
                    u = p1.tile([P, BL], BF16, tag="n1u", bufs=3, name=f"u{nt}")
                    nc.vector.tensor_tensor(u[:], h1raw[nt][:], invB1[:], op=ALU.mult)
                    v_ = p1.tile([P, BL], BF16, tag="n1v", bufs=3, name=f"v{nt}")
                    nc.vector.tensor_tensor(v_[:], u[:], nmuB1[:], op=ALU.add)
                    hn = p1.tile([P, BL], BF16, tag="n1h", bufs=3, name=f"hn{nt}")
                    nc.scalar.activation(hn[:], v_[:], AF.Relu,
                                         scale=n1s[:, nt:nt + 1], bias=n1b[:, nt:nt + 1])
                    nc.sync.dma_start(ag_h1_in[nt * P:(nt + 1) * P, :], hn[:])
                    if DEBUG_TAPS and nt == 3:
                        hf = p1.tile([P, BL], F32, tag="tapf", bufs=1, name="hf")
                        nc.vector.tensor_copy(hf[:], hn[:])
                        nc.sync.dma_start(taps["h1"][:], hf[:])

                nc.gpsimd.collective_compute(
                    "AllGather", ALU.bypass, replica_groups=RG,
                    ins=[ag_h1_in.opt()], outs=[ag_h1_out.opt()])

            # ================= phase 2: fc2 + LN2 + mixture ================
            with (
                tc.tile_pool(name="p2w", bufs=1) as p2w,
                tc.tile_pool(name="p2b", bufs=1) as p2b,
            ):
                w2sb = [None] * NKT

                def stage_A(r):
                    ps2 = [None] * NNT
                    for half in range(2):
                        nlo = half * (NNT // 2)
                        for n in range(nlo, nlo + NNT // 2):
                            ps2[n] = ppa.tile([P, BL], F32, tag="psa",
                                              name=f"ps2_{r}_{n}")
                        for k in range(NKT):
                            if r == 0 and half == 0:
                                w2t = p2w.tile([P, DL], BF16, tag=f"w2b{k}", bufs=1,
                                               name=f"w2b{k}")
                                for h in range(2):
                                    w2f = p2w.tile([P, DL // 2], F32, tag="w2f", bufs=3,
                                                   name=f"w2f{k}_{h}")
                                    nc.sync.dma_start(
                                        w2f[:], w2_ext[k * P:(k + 1) * P,
                                                       h * (DL // 2):(h + 1) * (DL // 2)])
                                    nc.vector.tensor_copy(
                                        w2t[:, h * (DL // 2):(h + 1) * (DL // 2)], w2f[:])
                                w2sb[k] = w2t
                            h1t = p2w.tile([P, BL], BF16, tag="h1s", bufs=4,
                                           name=f"h1t{r}_{half}_{k}")
                            nc.sync.dma_start(
                                h1t[:],
                                ag_h1_out[r * D + k * P: r * D + (k + 1) * P, :])
                            for n in range(nlo, nlo + NNT // 2):
                                nc.tensor.matmul(ps2[n][:],
                                                 w2sb[k][:, n * P:(n + 1) * P], h1t[:],
                                                 start=(k == 0), stop=(k == NKT - 1))
                    h2r = p2b.tile([P, NNT * BL], BF16, tag="h2raw", bufs=2,
                                   name=f"h2r{r}")
                    st2x = pps.tile([1, BL], F32, tag="pss", name=f"st2x{r}")
                    st2q = pps.tile([1, BL], F32, tag="pss", name=f"st2q{r}")
                    for n in range(NNT):
                        nc.scalar.activation(h2r[:, n * BL:(n + 1) * BL], ps2[n][:],
                                             AF.Identity, bias=fc2b[:, n:n + 1])
                        sq = p2b.tile([P, BL], BF16, tag="sq2", bufs=2,
                                      name=f"sq2_{r}_{n}")
                        nc.scalar.activation(sq[:], h2r[:, n * BL:(n + 1) * BL], AF.Square)
                        nc.tensor.matmul(st2x[:], ones_col_b[:],
                                         h2r[:, n * BL:(n + 1) * BL],
                                         start=(n == 0), stop=(n == NNT - 1))
                        nc.tensor.matmul(st2q[:], ones_col_b[:], sq[:],
                                         start=(n == 0), stop=(n == NNT - 1))
                    sx = p2b.tile([1, BL], F32, tag="ln2v", bufs=6, name=f"sx{r}")
                    nc.vector.tensor_copy(sx[:], st2x[:])
                    sq_ = p2b.tile([1, BL], F32, tag="ln2v", bufs=6, name=f"sqe{r}")
                    nc.vector.tensor_copy(sq_[:], st2q[:])
                    nc.sync.dma_start(st_in[r][0:1, :], sx[:])
                    nc.sync.dma_start(st_in[r][1:2, :], sq_[:])
                    nc.gpsimd.collective_compute(
                        "AllReduce", ALU.add, replica_groups=RG,
                        ins=[st_in[r].opt()], outs=[st_out[r].opt()])
                    return h2r

                def stage_B(r, h2r):
                    stx = p2b.tile([1, BL], F32, tag="ln2v", bufs=6, name=f"stxb{r}")
                    nc.sync.dma_start(stx[:], st_out[r][0:1, :])
                    stq = p2b.tile([1, BL], F32, tag="ln2v", bufs=6, name=f"stqb{r}")
                    nc.sync.dma_start(stq[:], st_out[r][1:2, :])
                    invB, nmuB = ln_stats_to_bcast(p2b, stx, stq, "ln2", f"B{r}")
                    scr = p2b.tile([M, BL], F32, tag="scr", bufs=1, name=f"scr{r}")
                    nc.sync.dma_start(scr[:], sc_out[r * M:(r + 1) * M, :])
                    scB_ps = ppb.tile([P, BL], F32, tag="psb", name=f"scBps{r}")
                    nc.tensor.matmul(scB_ps[:], Emat[:], scr[:], start=True, stop=True)
                    scB = p2b.tile([P, BL], BF16, tag="scB", bufs=2, name=f"scB{r}")
                    nc.scalar.activation(scB[:], scB_ps[:], AF.Copy)
                    mix_ps = ppb.tile([HL, BL], F32, tag="psb", name=f"mixps{r}")
                    for n in range(NNT):
                        u = p2b.tile([P, BL], BF16, tag="n2u", bufs=2, name=f"u2_{r}_{n}")
                        nc.vector.tensor_tensor(u[:], h2r[:, n * BL:(n + 1) * BL],
                                                invB[:], op=ALU.mult)
                        v_ = p2b.tile([P, BL], BF16, tag="n2v", bufs=2, name=f"v2_{r}_{n}")
                        nc.vector.tensor_tensor(v_[:], u[:], nmuB[:], op=ALU.add)
                        w_ = p2b.tile([P, BL], BF16, tag="n2w", bufs=2, name=f"w2_{r}_{n}")
                        nc.scalar.activation(w_[:], v_[:], AF.Relu,
                                             scale=n2s[:, n:n + 1], bias=n2b[:, n:n + 1])
                        pr = p2b.tile([P, BL], BF16, tag="n2p", bufs=2, name=f"p2_{r}_{n}")
                        nc.vector.tensor_tensor(pr[:], w_[:], scB[:], op=ALU.mult)
                        nc.tensor.matmul(mix_ps[:], Smat[:, n * HL:(n + 1) * HL], pr[:],
                                         start=(n == 0), stop=(n == NNT - 1))
                    mx = p2b.tile([HL, BL], F32, tag="mx", bufs=2, name=f"mx{r}")
                    nc.scalar.activation(mx[:], mix_ps[:], AF.Copy)
                    if DEBUG_TAPS and r == 0:
                        nc.sync.dma_start(taps["mixed"][:], mx[:])
                    mp = ppb.tile([ACT_DIM, BL], F32, tag="psb", name=f"mp{r}")
                    nc.tensor.matmul(mp[:], mwt[:], mx[:], start=True, stop=True)
                    hpm = p2b.tile([ACT_DIM, BL], F32, tag="hpm", bufs=2, name=f"hpm{r}")
                    nc.vector.tensor_copy(hpm[:], mp[:])
                    nc.sync.dma_start(hd_in[0:ACT_DIM, r * BL:(r + 1) * BL], hpm[:])
                    lp = ppb.tile([ACT_DIM, BL], F32, tag="psb", name=f"lp{r}")
                    nc.tensor.matmul(lp[:], lwt[:], mx[:], start=True, stop=True)
                    hpl = p2b.tile([ACT_DIM, BL], F32, tag="hpl", bufs=2, name=f"hpl{r}")
                    nc.vector.tensor_copy(hpl[:], lp[:])
                    nc.sync.dma_start(hd_in[ACT_DIM:2 * ACT_DIM, r * BL:(r + 1) * BL],
                                      hpl[:])

                h2_prev = stage_A(0)
                for r in range(1, NRB):
                    h2_cur = stage_A(r)
                    stage_B(r - 1, h2_prev)
                    h2_prev = h2_cur
                stage_B(NRB - 1, h2_prev)

                nc.gpsimd.collective_compute(
                    "AllReduce", ALU.add, replica_groups=RG,
                    ins=[hd_in.opt()], outs=[hd_out.opt()])

            # ================= phase 3: epilogue ===========================
            with tc.tile_pool(name="p3", bufs=1) as p3:
                for bb in range(NRB):
                    hm = p3.tile([ACT_DIM, BL], F32, tag="hm", bufs=2, name=f"hm{bb}")
                    nc.sync.dma_start(hm[:], hd_out[0:ACT_DIM, bb * BL:(bb + 1) * BL])
                    mfin = p3.tile([ACT_DIM, BL], F32, tag="mf", bufs=2, name=f"mf{bb}")
                    nc.scalar.activation(mfin[:], hm[:], AF.Identity, bias=mbt[:])
                    nc.sync.dma_start(out_ext[0:ACT_DIM, bb * BL:(bb + 1) * BL], mfin[:])
                    hl = p3.tile([ACT_DIM, BL], F32, tag="hlr", bufs=2, name=f"hl{bb}")
                    nc.sync.dma_start(hl[:], hd_out[ACT_DIM:2 * ACT_DIM,
                                                    bb * BL:(bb + 1) * BL])
                    th = p3.tile([ACT_DIM, BL], F32, tag="th", bufs=2, name=f"th{bb}")
                    nc.scalar.activation(th[:], hl[:], AF.Tanh, bias=lbt[:])
                    lfin = p3.tile([ACT_DIM, BL], F32, tag="lf", bufs=2, name=f"lf{bb}")
                    nc.vector.tensor_scalar(
                        lfin[:], th[:], 0.5 * (LOG_STD_MAX - LOG_STD_MIN),
                        LOG_STD_MIN + 0.5 * (LOG_STD_MAX - LOG_STD_MIN),
                        op0=ALU.mult, op1=ALU.add)
                    nc.sync.dma_start(out_ext[ACT_DIM:2 * ACT_DIM,
                                              bb * BL:(bb + 1) * BL], lfin[:])

    nc.compile()
    return nc


_NC_CACHE = None


def _get_nc():
    global _NC_CACHE
    if _NC_CACHE is None:
        _NC_CACHE = build_kernel()
    return _NC_CACHE


def make_in_maps(inputs):
    def f32c(a):
        return np.ascontiguousarray(np.asarray(a, np.float32))

    x = f32c(inputs["x"])
    fc2_W = f32c(inputs["fc2_W"])
    in_maps = []
    for i in range(N_CORES):
        in_maps.append({
            "x": np.ascontiguousarray(x[i * BL:(i + 1) * BL]),
            "gate_W": f32c(inputs["gate_W"]),
            "gate_b": f32c(inputs["gate_b"]),
            "fc1_W": f32c(inputs["fc1_W"]),
            "fc1_b": f32c(inputs["fc1_b"]),
            "norm1_scale": f32c(inputs["norm1_scale"]),
            "norm1_bias": f32c(inputs["norm1_bias"]),
            "fc2_W": np.ascontiguousarray(fc2_W[:, i * DL:(i + 1) * DL]),
            "fc2_b": f32c(inputs["fc2_b"])[i * DL:(i + 1) * DL].copy(),
            "norm2_scale": f32c(inputs["norm2_scale"])[i * DL:(i + 1) * DL].copy(),
            "norm2_bias": f32c(inputs["norm2_bias"])[i * DL:(i + 1) * DL].copy(),
            "mean_W": f32c(inputs["mean_W"])[i * HL:(i + 1) * HL, :].copy(),
            "mean_b": f32c(inputs["mean_b"]),
            "logstd_W": f32c(inputs["logstd_W"])[i * HL:(i + 1) * HL, :].copy(),
            "logstd_b": f32c(inputs["logstd_b"]),
        })
    return in_maps


def kernel(**inputs):
    topk = int(inputs.get("topk", TOPK))
    assert topk == TOPK, f"kernel compiled for topk={TOPK}, got {topk}"
    nc = _get_nc()
    in_maps = make_in_maps(inputs)
    res = run_bass_kernel_spmd(nc, in_maps, core_ids=list(range(N_CORES)))
    out = res.results[0]["out"]  # [64, B]; identical on every core
    mean = np.ascontiguousarray(out[:ACT_DIM, :].T)
    log_std = np.ascontiguousarray(out[ACT_DIM:, :].T)
    return mean, log_std


# revision 9
# speedup vs baseline: 1.0226x; 1.0226x over previous
"""Trainium2 Bass kernel for the MoE-routing Actor network (8 NeuronCores).

Sharding (per core i of 8):
  - Data-parallel gate/top-k + fc1 + LayerNorm1 + ReLU on the core's batch
    shard (512 rows), producing h1^T feature-major [8192, 512] bf16.
  - AllGather h1^T (batch-block concat) -> every core sees all 4096 rows.
  - Tensor-parallel fc2: core i holds fc2_W[:, i*1024:(i+1)*1024] bf16
    SBUF-resident; computes its 1024-feature slice for all batch in 8
    batch blocks of 512, LN2 stats AllReduced per block (pipelined).
  - LN2 + ReLU + expert-score multiply + group-of-16 mean via a constant
    selection matmul -> mixed^T slice [64, 512] per block, head partials,
    one AllReduce of head outputs [64, 4096], tanh/affine epilogue.
  - Output [64, 4096] (mean rows 0:32, log_std rows 32:64), host transposes.

All heavy matmuls run in bf16 (fp32 PE rate is 1/4 of bf16 on trn2).
"""

import numpy as np

import concourse.bass as bass
import concourse.bacc as bacc
import concourse.mybir as mybir
import concourse.tile as tile
from concourse.bass_utils import run_bass_kernel_spmd

F32 = mybir.dt.float32
BF16 = mybir.dt.bfloat16
AF = mybir.ActivationFunctionType
ALU = mybir.AluOpType
AX = mybir.AxisListType

N_CORES = 8
B, OBS, ACT_DIM, H, M, TOPK = 4096, 256, 32, 512, 16, 4
D = H * M          # 8192 trunk width
BL = B // N_CORES  # 512 local batch rows
DL = D // N_CORES  # 1024 local fc2 output features
HL = H // N_CORES  # 64 local mixed features
P = 128
NKT = D // P       # 64 k tiles over trunk width
NNT = DL // P      # 8 n tiles of local fc2 features
NBT = BL // P      # 4 batch tiles of the local shard
NRB = N_CORES      # 8 batch blocks of 512 in fc2 phase
LN_EPS = 1e-5
LOG_STD_MAX, LOG_STD_MIN = 2.0, -5.0
RG = [list(range(N_CORES))]

DEBUG_TAPS = False


def _consts():
    ident = np.eye(P, dtype=np.float32)
    ones_col = np.ones((P, 1), dtype=np.float32)
    ones_row = np.ones((1, P), dtype=np.float32)
    # E[k, p] = 1 if p % 16 == k  (broadcast 16 score rows over 128 partitions)
    E = np.zeros((M, P), dtype=np.float32)
    for p in range(P):
        E[p % M, p] = 1.0
    # S[n][p, g] = 1/16 if g == n*8 + p//16  (group-of-16 mean, n-th tile)
    S_all = np.zeros((NNT, P, HL), dtype=np.float32)
    for n in range(NNT):
        for p in range(P):
            S_all[n, p, n * (P // M) + p // M] = 1.0 / M
    return ident, ones_col, ones_row, E, S_all


def build_kernel():
    nc = bacc.Bacc(None, target_bir_lowering=False, num_devices=N_CORES)

    x_ext = nc.declare_dram_parameter("x", [BL, OBS], F32, isOutput=False)
    gw_ext = nc.declare_dram_parameter("gate_W", [OBS, M], F32, isOutput=False)
    gb_ext = nc.declare_dram_parameter("gate_b", [M], F32, isOutput=False)
    w1_ext = nc.declare_dram_parameter("fc1_W", [OBS, D], F32, isOutput=False)
    b1_ext = nc.declare_dram_parameter("fc1_b", [D], F32, isOutput=False)
    n1s_ext = nc.declare_dram_parameter("norm1_scale", [D], F32, isOutput=False)
    n1b_ext = nc.declare_dram_parameter("norm1_bias", [D], F32, isOutput=False)
    w2_ext = nc.declare_dram_parameter("fc2_W", [D, DL], F32, isOutput=False)
    b2_ext = nc.declare_dram_parameter("fc2_b", [DL], F32, isOutput=False)
    n2s_ext = nc.declare_dram_parameter("norm2_scale", [DL], F32, isOutput=False)
    n2b_ext = nc.declare_dram_parameter("norm2_bias", [DL], F32, isOutput=False)
    mw_ext = nc.declare_dram_parameter("mean_W", [HL, ACT_DIM], F32, isOutput=False)
    mb_ext = nc.declare_dram_parameter("mean_b", [ACT_DIM], F32, isOutput=False)
    lw_ext = nc.declare_dram_parameter("logstd_W", [HL, ACT_DIM], F32, isOutput=False)
    lb_ext = nc.declare_dram_parameter("logstd_b", [ACT_DIM], F32, isOutput=False)
    out_ext = nc.declare_dram_parameter("out", [2 * ACT_DIM, B], F32, isOutput=True)
    taps = {}
    if DEBUG_TAPS:
        taps["scores"] = nc.declare_dram_parameter("tap_scores", [M, BL], F32, isOutput=True)
        taps["h1"] = nc.declare_dram_parameter("tap_h1", [P, BL], F32, isOutput=True)
        taps["mixed"] = nc.declare_dram_parameter("tap_mixed", [HL, BL], F32, isOutput=True)

    ident_np, ones_col_np, ones_row_np, E_np, S_np = _consts()
    ident_dram = nc.inline_tensor(ident_np, name="ident")
    ones_col_dram = nc.inline_tensor(ones_col_np, name="ones_col")
    ones_row_dram = nc.inline_tensor(ones_row_np, name="ones_row")
    E_dram = nc.inline_tensor(E_np, name="Emat")
    S_flat = np.ascontiguousarray(S_np.transpose(1, 0, 2).reshape(P, NNT * HL))
    S_dram = nc.inline_tensor(S_flat, name="Smat")

    with tile.TileContext(nc) as tc:
        with (
            tc.tile_pool(name="cst", bufs=1) as cst,
            tc.tile_pool(name="dram", bufs=1, space="DRAM") as dram,
            tc.tile_pool(name="ppa", bufs=4, space="PSUM") as ppa,
            tc.tile_pool(name="pps", bufs=2, space="PSUM") as pps,
            tc.tile_pool(name="ppb", bufs=2, space="PSUM") as ppb,
        ):
            # ---------------- constants / small parameters ----------------
            ident = cst.tile([P, P], F32)
            nc.sync.dma_start(ident[:], ident_dram[:])
            ones_col_f = cst.tile([P, 1], F32)
            nc.sync.dma_start(ones_col_f[:], ones_col_dram[:])
            ones_col_b = cst.tile([P, 1], BF16)
            nc.vector.tensor_copy(ones_col_b[:], ones_col_f[:])
            ones_row_f = cst.tile([1, P], F32)
            nc.sync.dma_start(ones_row_f[:], ones_row_dram[:])
            ones_row_b = cst.tile([1, P], BF16)
            nc.vector.tensor_copy(ones_row_b[:], ones_row_f[:])
            Emat = cst.tile([M, P], F32)
            nc.sync.dma_start(Emat[:], E_dram[:])
            Smat = cst.tile([P, NNT * HL], BF16)

            fc1b = cst.tile([P, NKT], F32)
            nc.sync.dma_start(fc1b[:], b1_ext.ap().rearrange("(a b) -> b a", b=P))
            n1s = cst.tile([P, NKT], F32)
            nc.sync.dma_start(n1s[:], n1s_ext.ap().rearrange("(a b) -> b a", b=P))
            n1b = cst.tile([P, NKT], F32)
            nc.sync.dma_start(n1b[:], n1b_ext.ap().rearrange("(a b) -> b a", b=P))
            fc2b = cst.tile([P, NNT], F32)
            nc.sync.dma_start(fc2b[:], b2_ext.ap().rearrange("(a b) -> b a", b=P))
            n2s = cst.tile([P, NNT], F32)
            nc.sync.dma_start(n2s[:], n2s_ext.ap().rearrange("(a b) -> b a", b=P))
            n2b = cst.tile([P, NNT], F32)
            nc.sync.dma_start(n2b[:], n2b_ext.ap().rearrange("(a b) -> b a", b=P))
            gbf = cst.tile([1, M], F32)
            nc.sync.dma_start(gbf[:], gb_ext.ap().rearrange("(a b) -> a b", a=1))
            mwt = cst.tile([HL, ACT_DIM], F32)
            nc.sync.dma_start(mwt[:], mw_ext[:])
            lwt = cst.tile([HL, ACT_DIM], F32)
            nc.sync.dma_start(lwt[:], lw_ext[:])
            mbt = cst.tile([ACT_DIM, 1], F32)
            nc.sync.dma_start(mbt[:], mb_ext.ap().rearrange("(a b) -> a b", b=1))
            lbt = cst.tile([ACT_DIM, 1], F32)
            nc.sync.dma_start(lbt[:], lb_ext.ap().rearrange("(a b) -> a b", b=1))
            gwf = cst.tile([P, 2 * M], F32)
            for kt in range(2):
                nc.sync.dma_start(gwf[:, kt * M:(kt + 1) * M],
                                  gw_ext[kt * P:(kt + 1) * P, :])

            eps_t = cst.tile([1, 1], F32)
            nc.any.memset(eps_t[:], LN_EPS)
            xT = cst.tile([P, 2 * BL], BF16)  # x^T k-tiles side by side [k, b]
            scT = cst.tile([M, BL], F32)      # local top-k scores^T

            # DRAM bounce buffers
            ag_h1_in = dram.tile([D, BL], BF16)
            ag_h1_out = dram.tile([N_CORES * D, BL], BF16, addr_space="Shared")
            sc_in = dram.tile([M, BL], F32)
            sc_out = dram.tile([N_CORES * M, BL], F32, addr_space="Shared")
            st_in = [dram.tile([2, BL], F32, tag=f"st_in{r}", name=f"st_in{r}")
                     for r in range(NRB)]
            st_out = [dram.tile([2, BL], F32, addr_space="Shared",
                                tag=f"st_out{r}", name=f"st_out{r}")
                      for r in range(NRB)]
            hd_in = dram.tile([2 * ACT_DIM, B], F32)
            hd_out = dram.tile([2 * ACT_DIM, B], F32, addr_space="Shared")

            def ln_stats_to_bcast(pool, stx, stq, tagp, un):
                """[1,BL] f32 sum/sumsq vectors -> bf16 bcast tiles invB, nmuB."""
                def v(nm):
                    return pool.tile([1, BL], F32, tag=f"{tagp}v", bufs=6, name=f"{nm}{un}")
                mu = v("mu")
                nc.vector.tensor_scalar_mul(mu[:], stx[:], 1.0 / D)
                e2 = v("e2")
                nc.vector.tensor_scalar_mul(e2[:], stq[:], 1.0 / D)
                mu2 = v("mu2")
                nc.scalar.activation(mu2[:], mu[:], AF.Square)
                var = v("var")
                nc.vector.tensor_tensor(var[:], e2[:], mu2[:], op=ALU.subtract)
                sd = v("sd")
                nc.scalar.activation(sd[:], var[:], AF.Sqrt, bias=eps_t[:])
                inv = v("inv")
                nc.vector.reciprocal(inv[:], sd[:])
                nmuinv = v("nmuinv")
                nc.vector.tensor_tensor(nmuinv[:], mu[:], inv[:], op=ALU.mult)
                nmu2 = v("nmu2")
                nc.vector.tensor_scalar_mul(nmu2[:], nmuinv[:], -1.0)
                invB_ps = ppb.tile([P, BL], F32, tag="psb", name=f"invBps{un}")
                nc.tensor.matmul(invB_ps[:], ones_row_f[:], inv[:], start=True, stop=True)
                invB = pool.tile([P, BL], BF16, tag=f"{tagp}bc", bufs=4, name=f"invB{un}")
                nc.scalar.activation(invB[:], invB_ps[:], AF.Copy)
                nmuB_ps = ppb.tile([P, BL], F32, tag="psb", name=f"nmuBps{un}")
                nc.tensor.matmul(nmuB_ps[:], ones_row_f[:], nmu2[:], start=True, stop=True)
                nmuB = pool.tile([P, BL], BF16, tag=f"{tagp}bc", bufs=4, name=f"nmuB{un}")
                nc.scalar.activation(nmuB[:], nmuB_ps[:], AF.Copy)
                return invB, nmuB

            # ================= phase 0 + 1 (pool p1) =======================
            with tc.tile_pool(name="p1", bufs=1) as p1:
                Smat_f = p1.tile([P, NNT * HL], F32)
                nc.sync.dma_start(Smat_f[:], S_dram[:])
                nc.vector.tensor_copy(Smat[:], Smat_f[:])
                # ---- x^T ----
                xTf = p1.tile([P, 2 * BL], F32, tag="xTf", bufs=1, name="xTf")
                for bt in range(NBT):
                    xl = p1.tile([P, OBS], F32, tag="xload", bufs=2, name=f"xl{bt}")
                    nc.sync.dma_start(xl[:], x_ext[bt * P:(bt + 1) * P, :])
                    for kt in range(2):
                        tp = ppb.tile([P, P], F32, tag="psb", name=f"xtp{bt}_{kt}")
                        nc.tensor.transpose(tp[:], xl[:, kt * P:(kt + 1) * P], ident[:])
                        nc.scalar.activation(
                            xTf[:, kt * BL + bt * P: kt * BL + (bt + 1) * P],
                            tp[:], AF.Copy)
                        nc.vector.tensor_copy(
                            xT[:, kt * BL + bt * P: kt * BL + (bt + 1) * P],
                            tp[:])

                # ---- gate + softmax + top-4 ----
                for bt in range(NBT):
                    gp = ppb.tile([P, M], F32, tag="psb", name=f"gp{bt}")
                    for kt in range(2):
                        nc.tensor.matmul(
                            gp[:], xTf[:, kt * BL + bt * P: kt * BL + (bt + 1) * P],
                            gwf[:, kt * M:(kt + 1) * M], start=(kt == 0), stop=False)
                    nc.tensor.matmul(gp[:], ones_row_f[:], gbf[:], start=False, stop=True)

                    def g1(nm):
                        return p1.tile([P, 1], F32, tag="gs1", bufs=6, name=f"{nm}{bt}")

                    def g16(nm):
                        return p1.tile([P, M], F32, tag="gs16", bufs=6, name=f"{nm}{bt}")

                    gmax = g1("gmax")
                    nc.vector.tensor_reduce(gmax[:], gp[:], AX.X, ALU.max)
                    ngmax = g1("ngmax")
                    nc.vector.tensor_scalar_mul(ngmax[:], gmax[:], -1.0)
                    ge = g16("ge")
                    nc.scalar.activation(ge[:], gp[:], AF.Exp, bias=ngmax[:])
                    gsum = g1("gsum")
                    nc.vector.reduce_sum(gsum[:], ge[:], axis=AX.X)
                    grec = g1("grec")
                    nc.vector.reciprocal(grec[:], gsum[:])
                    s0 = g16("s0")
                    nc.vector.tensor_scalar_mul(s0[:], ge[:], grec[:])
                    mt4 = p1.tile([P, TOPK], F32, tag="gs4", bufs=2, name=f"mt4{bt}")
                    w = s0
                    for t in range(TOPK):
                        nc.vector.tensor_reduce(mt4[:, t:t + 1], w[:], AX.X, ALU.max)
                        if t < TOPK - 1:
                            msk = g16(f"msk{t}_")
                            nc.vector.tensor_scalar(msk[:], w[:], mt4[:, t:t + 1], None,
                                                    op0=ALU.is_ge)
                            w2_ = g16(f"w{t}_")
                            nc.vector.tensor_tensor(w2_[:], w[:], msk[:], op=ALU.subtract)
                            w = w2_
                    tsum = g1("tsum")
                    nc.vector.reduce_sum(tsum[:], mt4[:], axis=AX.X)
                    trec = g1("trec")
                    nc.vector.reciprocal(trec[:], tsum[:])
                    keep = g16("keep")
                    nc.vector.tensor_scalar(keep[:], s0[:], mt4[:, TOPK - 1:TOPK], None,
                                            op0=ALU.is_ge)
                    sn = g16("sn")
                    nc.vector.tensor_scalar_mul(sn[:], s0[:], trec[:])
                    sc = g16("sc")
                    nc.vector.tensor_tensor(sc[:], sn[:], keep[:], op=ALU.mult)
                    stp = ppb.tile([M, P], F32, tag="psb", name=f"stp{bt}")
                    nc.tensor.transpose(stp[:], sc[:], ident[:])
                    nc.scalar.activation(scT[:, bt * P:(bt + 1) * P], stp[:], AF.Copy)
                nc.sync.dma_start(sc_in[:], scT[:])
                nc.gpsimd.collective_compute(
                    "AllGather", ALU.bypass, replica_groups=RG,
                    ins=[sc_in.opt()], outs=[sc_out.opt()])
                if DEBUG_TAPS:
                    nc.sync.dma_start(taps["scores"][:], scT[:])

                # ---- fc1 (bf16) + LN1 stats ----
                w1b = []
                for kt in range(2):
                    w1t = p1.tile([P, D], BF16, tag=f"w1b{kt}", bufs=1, name=f"w1b{kt}")
                    for h in range(2):
                        w1f = p1.tile([P, D // 2], F32, tag="w1f", bufs=2,
                                      name=f"w1f{kt}_{h}")
                        nc.sync.dma_start(
                            w1f[:], w1_ext[kt * P:(kt + 1) * P,
                                           h * (D // 2):(h + 1) * (D // 2)])
                        nc.vector.tensor_copy(w1t[:, h * (D // 2):(h + 1) * (D // 2)],
                                              w1f[:])
                    w1b.append(w1t)

                h1raw = []
                st1x = pps.tile([1, BL], F32, tag="pss", name="st1x")
                st1q = pps.tile([1, BL], F32, tag="pss", name="st1q")
                for nt in range(NKT):
                    ps1 = ppa.tile([P, BL], F32, tag="psa", name=f"ps1_{nt}")
                    for kt in range(2):
                        nc.tensor.matmul(ps1[:], w1b[kt][:, nt * P:(nt + 1) * P],
                                         xT[:, kt * BL:(kt + 1) * BL],
                                         start=(kt == 0), stop=(kt == 1))
                    h1r = p1.tile([P, BL], BF16, tag=f"h1raw{nt}", bufs=1,
                                  name=f"h1r{nt}")
                    nc.scalar.activation(h1r[:], ps1[:], AF.Identity,
                                         bias=fc1b[:, nt:nt + 1])
                    h1raw.append(h1r)
                    sq = p1.tile([P, BL], BF16, tag="sq1", bufs=3, name=f"sq1_{nt}")
                    nc.scalar.activation(sq[:], h1r[:], AF.Square)
                    nc.tensor.matmul(st1x[:], ones_col_b[:], h1r[:],
                                     start=(nt == 0), stop=(nt == NKT - 1))
                    nc.tensor.matmul(st1q[:], ones_col_b[:], sq[:],
                                     start=(nt == 0), stop=(nt == NKT - 1))

                sx1 = p1.tile([1, BL], F32, tag="ln1v", bufs=6, name="sx1")
                nc.vector.tensor_copy(sx1[:], st1x[:])
                sq1v = p1.tile([1, BL], F32, tag="ln1v", bufs=6, name="sq1v")
                nc.vector.tensor_copy(sq1v[:], st1q[:])
                invB1, nmuB1 = ln_stats_to_bcast(p1, sx1, sq1v, "ln1", "L1")

                # ---- LN1 normalize + relu -> AG input ----
                for nt in range(NKT):
                    u = p1.tile([P, BL], BF16, tag="n1u", bufs=3, name=f"u{nt}")
                    nc.vector.tensor_tensor(u[:], h1raw[nt][:], invB1[:], op=ALU.mult)
                    v_ = p1.tile([P, BL], BF16, tag="n1v", bufs=3, name=f"v{nt}")
                    nc.vector.tensor_tensor(v_[:], u[:], nmuB1[:], op=ALU.add)
                    hn = p1.tile([P, BL], BF16, tag="n1h", bufs=3, name=f"hn{nt}")
                    nc.scalar.activation(hn[:], v_[:], AF.Relu,
                                         scale=n1s[:, nt:nt + 1], bias=n1b[:, nt:nt + 1])
                    nc.sync.dma_start(ag_h1_in[nt * P:(nt + 1) * P, :], hn[:])
                    if DEBUG_TAPS and nt == 3:
                        hf = p1.tile([P, BL], F32, tag="tapf", bufs=1, name="hf")
                        nc.vector.tensor_copy(hf[:], hn[:])
                        nc.sync.dma_start(taps["h1"][:], hf[:])

                nc.gpsimd.collective_compute(
                    "AllGather", ALU.bypass, replica_groups=RG,
                    ins=[ag_h1_in.opt()], outs=[ag_h1_out.opt()])

            # ================= phase 2: fc2 + LN2 + mixture ================
            with (
                tc.tile_pool(name="p2w", bufs=1) as p2w,
                tc.tile_pool(name="p2b", bufs=1) as p2b,
            ):
                w2sb = [None] * NKT

                def stage_A(r):
                    ps2 = [None] * NNT
                    for half in range(2):
                        nlo = half * (NNT // 2)
                        for n in range(nlo, nlo + NNT // 2):
                            ps2[n] = ppa.tile([P, BL], F32, tag="psa",
                                              name=f"ps2_{r}_{n}")
                        for k in range(NKT):
                            if r == 0 and half == 0:
                                w2t = p2w.tile([P, DL], BF16, tag=f"w2b{k}", bufs=1,
                                               name=f"w2b{k}")
                                for h in range(2):
                                    w2f = p2w.tile([P, DL // 2], F32, tag="w2f", bufs=3,
                                                   name=f"w2f{k}_{h}")
                                    nc.sync.dma_start(
                                        w2f[:], w2_ext[k * P:(k + 1) * P,
                                                       h * (DL // 2):(h + 1) * (DL // 2)])
                                    nc.vector.tensor_copy(
                                        w2t[:, h * (DL // 2):(h + 1) * (DL // 2)], w2f[:])
                                w2sb[k] = w2t
                            h1t = p2w.tile([P, BL], BF16, tag="h1s", bufs=4,
                                           name=f"h1t{r}_{half}_{k}")
                            nc.sync.dma_start(
                                h1t[:],
                                ag_h1_out[r * D + k * P: r * D + (k + 1) * P, :])
                            for n in range(nlo, nlo + NNT // 2):
                                nc.tensor.matmul(ps2[n][:],
                                                 w2sb[k][:, n * P:(n + 1) * P], h1t[:],
                                                 start=(k == 0), stop=(k == NKT - 1))
                    h2r = p2b.tile([P, NNT * BL], BF16, tag="h2raw", bufs=2,
                                   name=f"h2r{r}")
                    st2x = pps.tile([1, BL], F32, tag="pss", name=f"st2x{r}")
                    st2q = pps.tile([1, BL], F32, tag="pss", name=f"st2q{r}")
                    for n in range(NNT):
                        nc.scalar.activation(h2r[:, n * BL:(n + 1) * BL], ps2[n][:],
                                             AF.Identity, bias=fc2b[:, n:n + 1])
                        sq = p2b.tile([P, BL], BF16, tag="sq2", bufs=2,
                                      name=f"sq2_{r}_{n}")
                        nc.scalar.activation(sq[:], h2r[:, n * BL:(n + 1) * BL], AF.Square)
                        nc.tensor.matmul(st2x[:], ones_col_b[:],
                                         h2r[:, n * BL:(n + 1) * BL],
                                         start=(n == 0), stop=(n == NNT - 1))
                        nc.tensor.matmul(st2q[:], ones_col_b[:], sq[:],
                                         start=(n == 0), stop=(n == NNT - 1))
                    sx = p2b.tile([1, BL], F32, tag="ln2v", bufs=6, name=f"sx{r}")
                    nc.vector.tensor_copy(sx[:], st2x[:])
                    sq_ = p2b.tile([1, BL], F32, tag="ln2v", bufs=6, name=f"sqe{r}")
                    nc.vector.tensor_copy(sq_[:], st2q[:])
                    nc.sync.dma_start(st_in[r][0:1, :], sx[:])
                    nc.sync.dma_start(st_in[r][1:2, :], sq_[:])
                    nc.gpsimd.collective_compute(
                        "AllReduce", ALU.add, replica_groups=RG,
                        ins=[st_in[r].opt()], outs=[st_out[r].opt()])
                    return h2r

                def stage_B(r, h2r):
                    stx = p2b.tile([1, BL], F32, tag="ln2v", bufs=6, name=f"stxb{r}")
                    nc.sync.dma_start(stx[:], st_out[r][0:1, :])
                    stq = p2b.tile([1, BL], F32, tag="ln2v", bufs=6, name=f"stqb{r}")
                    nc.sync.dma_start(stq[:], st_out[r][1:2, :])
                    invB, nmuB = ln_stats_to_bcast(p2b, stx, stq, "ln2", f"B{r}")
                    scr = p2b.tile([M, BL], F32, tag="scr", bufs=1, name=f"scr{r}")
                    nc.sync.dma_start(scr[:], sc_out[r * M:(r + 1) * M, :])
                    scB_ps = ppb.tile([P, BL], F32, tag="psb", name=f"scBps{r}")
                    nc.tensor.matmul(scB_ps[:], Emat[:], scr[:], start=True, stop=True)
                    scB = p2b.tile([P, BL], BF16, tag="scB", bufs=2, name=f"scB{r}")
                    nc.scalar.activation(scB[:], scB_ps[:], AF.Copy)
                    mix_ps = ppb.tile([HL, BL], F32, tag="psb", name=f"mixps{r}")
                    for n in range(NNT):
                        u = p2b.tile([P, BL], BF16, tag="n2u", bufs=2, name=f"u2_{r}_{n}")
                        nc.vector.tensor_tensor(u[:], h2r[:, n * BL:(n + 1) * BL],
                                                invB[:], op=ALU.mult)
                        v_ = p2b.tile([P, BL], BF16, tag="n2v", bufs=2, name=f"v2_{r}_{n}")
                        nc.vector.tensor_tensor(v_[:], u[:], nmuB[:], op=ALU.add)
                        w_ = p2b.tile([P, BL], BF16, tag="n2w", bufs=2, name=f"w2_{r}_{n}")
                        nc.scalar.activation(w_[:], v_[:], AF.Relu,
                                             scale=n2s[:, n:n + 1], bias=n2b[:, n:n + 1])
                        pr = p2b.tile([P, BL], BF16, tag="n2p", bufs=2, name=f"p2_{r}_{n}")
                        nc.vector.tensor_tensor(pr[:], w_[:], scB[:], op=ALU.mult)
                        nc.tensor.matmul(mix_ps[:], Smat[:, n * HL:(n + 1) * HL], pr[:],
                                         start=(n == 0), stop=(n == NNT - 1))
                    mx = p2b.tile([HL, BL], F32, tag="mx", bufs=2, name=f"mx{r}")
                    nc.scalar.activation(mx[:], mix_ps[:], AF.Copy)
                    if DEBUG_TAPS and r == 0:
                        nc.sync.dma_start(taps["mixed"][:], mx[:])
                    mp = ppb.tile([ACT_DIM, BL], F32, tag="psb", name=f"mp{r}")
                    nc.tensor.matmul(mp[:], mwt[:], mx[:], start=True, stop=True)
                    hpm = p2b.tile([ACT_DIM, BL], F32, tag="hpm", bufs=2, name=f"hpm{r}")
                    nc.vector.tensor_copy(hpm[:], mp[:])
                    nc.sync.dma_start(hd_in[0:ACT_DIM, r * BL:(r + 1) * BL], hpm[:])
                    lp = ppb.tile([ACT_DIM, BL], F32, tag="psb", name=f"lp{r}")
                    nc.tensor.matmul(lp[:], lwt[:], mx[:], start=True, stop=True)
                    hpl = p2b.tile([ACT_DIM, BL], F32, tag="hpl", bufs=2, name=f"hpl{r}")
                    nc.vector.tensor_copy(hpl[:], lp[:])
                    nc.sync.dma_start(hd_in[ACT_DIM:2 * ACT_DIM, r * BL:(r + 1) * BL],
                                      hpl[:])

                h2_prev = stage_A(0)
                for r in range(1, NRB):
                    h2_cur = stage_A(r)
                    stage_B(r - 1, h2_prev)
                    h2_prev = h2_cur
                stage_B(NRB - 1, h2_prev)

                nc.gpsimd.collective_compute(
                    "AllReduce", ALU.add, replica_groups=RG,
                    ins=[hd_in.opt()], outs=[hd_out.opt()])

            # ================= phase 3: epilogue ===========================
            with tc.tile_pool(name="p3", bufs=1) as p3:
                for bb in range(NRB):
                    hm = p3.tile([ACT_DIM, BL], F32, tag="hm", bufs=2, name=f"hm{bb}")
                    nc.sync.dma_start(hm[:], hd_out[0:ACT_DIM, bb * BL:(bb + 1) * BL])
                    mfin = p3.tile([ACT_DIM, BL], F32, tag="mf", bufs=2, name=f"mf{bb}")
                    nc.scalar.activation(mfin[:], hm[:], AF.Identity, bias=mbt[:])
                    nc.sync.dma_start(out_ext[0:ACT_DIM, bb * BL:(bb + 1) * BL], mfin[:])
                    hl = p3.tile([ACT_DIM, BL], F32, tag="hlr", bufs=2, name=f"hl{bb}")
                    nc.sync.dma_start(hl[:], hd_out[ACT_DIM:2 * ACT_DIM,
                                                    bb * BL:(bb + 1) * BL])
                    th = p3.tile([ACT_DIM, BL], F32, tag="th", bufs=2, name=f"th{bb}")
                    nc.scalar.activation(th[:], hl[:], AF.Tanh, bias=lbt[:])
                    lfin = p3.tile([ACT_DIM, BL], F32, tag="lf", bufs=2, name=f"lf{bb}")
                    nc.vector.tensor_scalar(
                        lfin[:], th[:], 0.5 * (LOG_STD_MAX - LOG_STD_MIN),
                        LOG_STD_MIN + 0.5 * (LOG_STD_MAX - LOG_STD_MIN),
                        op0=ALU.mult, op1=ALU.add)
                    nc.sync.dma_start(out_ext[ACT_DIM:2 * ACT_DIM,
                                              bb * BL:(bb + 1) * BL], lfin[:])

    nc.compile()
    return nc


_NC_CACHE = None


def _get_nc():
    global _NC_CACHE
    if _NC_CACHE is None:
        _NC_CACHE = build_kernel()
    return _NC_CACHE


def make_in_maps(inputs):
    def f32c(a):
        return np.ascontiguousarray(np.asarray(a, np.float32))

    x = f32c(inputs["x"])
    fc2_W = f32c(inputs["fc2_W"])
    in_maps = []
    for i in range(N_CORES):
        in_maps.append({
            "x": np.ascontiguousarray(x[i * BL:(i + 1) * BL]),
            "gate_W": f32c(inputs["gate_W"]),
            "gate_b": f32c(inputs["gate_b"]),
            "fc1_W": f32c(inputs["fc1_W"]),
            "fc1_b": f32c(inputs["fc1_b"]),
            "norm1_scale": f32c(inputs["norm1_scale"]),
            "norm1_bias": f32c(inputs["norm1_bias"]),
            "fc2_W": np.ascontiguousarray(fc2_W[:, i * DL:(i + 1) * DL]),
            "fc2_b": f32c(inputs["fc2_b"])[i * DL:(i + 1) * DL].copy(),
            "norm2_scale": f32c(inputs["norm2_scale"])[i * DL:(i + 1) * DL].copy(),
            "norm2_bias": f32c(inputs["norm2_bias"])[i * DL:(i + 1) * DL].copy(),
            "mean_W": f32c(inputs["mean_W"])[i * HL:(i + 1) * HL, :].copy(),
            "mean_b": f32c(inputs["mean_b"]),
            "logstd_W": f32c(inputs["logstd_W"])[i * HL:(i + 1) * HL, :].copy(),
            "logstd_b": f32c(inputs["logstd_b"]),
        })
    return in_maps


def kernel(**inputs):
    topk = int(inputs.get("topk", TOPK))
    assert topk == TOPK, f"kernel compiled for topk={TOPK}, got {topk}"
    nc = _get_nc()
    in_maps = make_in_maps(inputs)
    res = run_bass_kernel_spmd(nc, in_maps, core_ids=list(range(N_CORES)))
    out = res.results[0]["out"]  # [64, B]; identical on every core
    mean = np.ascontiguousarray(out[:ACT_DIM, :].T)
    log_std = np.ascontiguousarray(out[ACT_DIM:, :].T)
    return mean, log_std


# revision 12
# speedup vs baseline: 1.0717x; 1.0480x over previous
"""Trainium2 Bass kernel for the MoE-routing Actor network (8 NeuronCores).

Sharding (per core i of 8):
  - Data-parallel gate/top-k + fc1 + LayerNorm1 + ReLU on the core's batch
    shard (512 rows), producing h1^T feature-major [8192, 512] bf16.
  - AllGather h1^T (batch-block concat) -> every core sees all 4096 rows.
  - Tensor-parallel fc2: core i holds fc2_W[:, i*1024:(i+1)*1024] bf16
    SBUF-resident; computes its 1024-feature slice for all batch in 8
    batch blocks of 512, LN2 stats AllReduced per block (pipelined).
  - LN2 + ReLU + expert-score multiply + group-of-16 mean via a constant
    selection matmul -> mixed^T slice [64, 512] per block, head partials,
    one AllReduce of head outputs [64, 4096], tanh/affine epilogue.
  - Output [64, 4096] (mean rows 0:32, log_std rows 32:64), host transposes.

All heavy matmuls run in bf16 (fp32 PE rate is 1/4 of bf16 on trn2).
"""

import numpy as np

import concourse.bass as bass
import concourse.bacc as bacc
import concourse.mybir as mybir
import concourse.tile as tile
from concourse.bass_utils import run_bass_kernel_spmd

F32 = mybir.dt.float32
BF16 = mybir.dt.bfloat16
AF = mybir.ActivationFunctionType
ALU = mybir.AluOpType
AX = mybir.AxisListType

N_CORES = 8
B, OBS, ACT_DIM, H, M, TOPK = 4096, 256, 32, 512, 16, 4
D = H * M          # 8192 trunk width
BL = B // N_CORES  # 512 local batch rows
DL = D // N_CORES  # 1024 local fc2 output features
HL = H // N_CORES  # 64 local mixed features
P = 128
NKT = D // P       # 64 k tiles over trunk width
NNT = DL // P      # 8 n tiles of local fc2 features
NBT = BL // P      # 4 batch tiles of the local shard
NRB = N_CORES      # 8 batch blocks of 512 in fc2 phase
LN_EPS = 1e-5
LOG_STD_MAX, LOG_STD_MIN = 2.0, -5.0
RG = [list(range(N_CORES))]

DEBUG_TAPS = False


def _consts():
    ident = np.eye(P, dtype=np.float32)
    ones_col = np.ones((P, 1), dtype=np.float32)
    ones_row = np.ones((1, P), dtype=np.float32)
    # E[k, p] = 1 if p % 16 == k  (broadcast 16 score rows over 128 partitions)
    E = np.zeros((M, P), dtype=np.float32)
    for p in range(P):
        E[p % M, p] = 1.0
    # S[n][p, g] = 1/16 if g == n*8 + p//16  (group-of-16 mean, n-th tile)
    S_all = np.zeros((NNT, P, HL), dtype=np.float32)
    for n in range(NNT):
        for p in range(P):
            S_all[n, p, n * (P // M) + p // M] = 1.0 / M
    return ident, ones_col, ones_row, E, S_all


def build_kernel():
    nc = bacc.Bacc(None, target_bir_lowering=False, num_devices=N_CORES)

    x_ext = nc.declare_dram_parameter("x", [BL, OBS], F32, isOutput=False)
    gw_ext = nc.declare_dram_parameter("gate_W", [OBS, M], F32, isOutput=False)
    gb_ext = nc.declare_dram_parameter("gate_b", [M], F32, isOutput=False)
    w1_ext = nc.declare_dram_parameter("fc1_W", [OBS, D], F32, isOutput=False)
    b1_ext = nc.declare_dram_parameter("fc1_b", [D], F32, isOutput=False)
    n1s_ext = nc.declare_dram_parameter("norm1_scale", [D], F32, isOutput=False)
    n1b_ext = nc.declare_dram_parameter("norm1_bias", [D], F32, isOutput=False)
    w2_ext = nc.declare_dram_parameter("fc2_W", [D, DL], F32, isOutput=False)
    b2_ext = nc.declare_dram_parameter("fc2_b", [DL], F32, isOutput=False)
    n2s_ext = nc.declare_dram_parameter("norm2_scale", [DL], F32, isOutput=False)
    n2b_ext = nc.declare_dram_parameter("norm2_bias", [DL], F32, isOutput=False)
    mw_ext = nc.declare_dram_parameter("mean_W", [HL, ACT_DIM], F32, isOutput=False)
    mb_ext = nc.declare_dram_parameter("mean_b", [ACT_DIM], F32, isOutput=False)
    lw_ext = nc.declare_dram_parameter("logstd_W", [HL, ACT_DIM], F32, isOutput=False)
    lb_ext = nc.declare_dram_parameter("logstd_b", [ACT_DIM], F32, isOutput=False)
    out_ext = nc.declare_dram_parameter("out", [2 * ACT_DIM, B], F32, isOutput=True)
    taps = {}
    if DEBUG_TAPS:
        taps["scores"] = nc.declare_dram_parameter("tap_scores", [M, BL], F32, isOutput=True)
        taps["h1"] = nc.declare_dram_parameter("tap_h1", [P, BL], F32, isOutput=True)
        taps["mixed"] = nc.declare_dram_parameter("tap_mixed", [HL, BL], F32, isOutput=True)

    ident_np, ones_col_np, ones_row_np, E_np, S_np = _consts()
    ident_dram = nc.inline_tensor(ident_np, name="ident")
    ones_col_dram = nc.inline_tensor(ones_col_np, name="ones_col")
    ones_row_dram = nc.inline_tensor(ones_row_np, name="ones_row")
    E_dram = nc.inline_tensor(E_np, name="Emat")
    S_flat = np.ascontiguousarray(S_np.transpose(1, 0, 2).reshape(P, NNT * HL))
    S_dram = nc.inline_tensor(S_flat, name="Smat")

    with tile.TileContext(nc) as tc:
        with (
            tc.tile_pool(name="cst", bufs=1) as cst,
            tc.tile_pool(name="dram", bufs=1, space="DRAM") as dram,
            tc.tile_pool(name="ppa", bufs=4, space="PSUM") as ppa,
            tc.tile_pool(name="pps", bufs=2, space="PSUM") as pps,
            tc.tile_pool(name="ppb", bufs=2, space="PSUM") as ppb,
        ):
            # ---------------- constants / small parameters ----------------
            ident = cst.tile([P, P], F32)
            nc.sync.dma_start(ident[:], ident_dram[:])
            ones_col_f = cst.tile([P, 1], F32)
            nc.sync.dma_start(ones_col_f[:], ones_col_dram[:])
            ones_col_b = cst.tile([P, 1], BF16)
            nc.vector.tensor_copy(ones_col_b[:], ones_col_f[:])
            ones_row_f = cst.tile([1, P], F32)
            nc.sync.dma_start(ones_row_f[:], ones_row_dram[:])
            ones_row_b = cst.tile([1, P], BF16)
            nc.vector.tensor_copy(ones_row_b[:], ones_row_f[:])
            Emat_f = cst.tile([M, P], F32)
            nc.sync.dma_start(Emat_f[:], E_dram[:])
            Emat = cst.tile([M, P], BF16)
            nc.vector.tensor_copy(Emat[:], Emat_f[:])
            Smat = cst.tile([P, NNT * HL], BF16)

            fc1b = cst.tile([P, NKT], F32)
            nc.sync.dma_start(fc1b[:], b1_ext.ap().rearrange("(a b) -> b a", b=P))
            n1s = cst.tile([P, NKT], F32)
            nc.sync.dma_start(n1s[:], n1s_ext.ap().rearrange("(a b) -> b a", b=P))
            n1b = cst.tile([P, NKT], F32)
            nc.sync.dma_start(n1b[:], n1b_ext.ap().rearrange("(a b) -> b a", b=P))
            fc2b = cst.tile([P, NNT], F32)
            nc.sync.dma_start(fc2b[:], b2_ext.ap().rearrange("(a b) -> b a", b=P))
            n2s = cst.tile([P, NNT], F32)
            nc.sync.dma_start(n2s[:], n2s_ext.ap().rearrange("(a b) -> b a", b=P))
            n2b = cst.tile([P, NNT], F32)
            nc.sync.dma_start(n2b[:], n2b_ext.ap().rearrange("(a b) -> b a", b=P))
            gbf = cst.tile([1, M], F32)
            nc.sync.dma_start(gbf[:], gb_ext.ap().rearrange("(a b) -> a b", a=1))
            hwt_f = cst.tile([HL, 2 * ACT_DIM], F32)
            nc.sync.dma_start(hwt_f[:, 0:ACT_DIM], mw_ext[:])
            nc.sync.dma_start(hwt_f[:, ACT_DIM:2 * ACT_DIM], lw_ext[:])
            hwt = cst.tile([HL, 2 * ACT_DIM], BF16)
            nc.vector.tensor_copy(hwt[:], hwt_f[:])
            mbt = cst.tile([ACT_DIM, 1], F32)
            nc.sync.dma_start(mbt[:], mb_ext.ap().rearrange("(a b) -> a b", b=1))
            lbt = cst.tile([ACT_DIM, 1], F32)
            nc.sync.dma_start(lbt[:], lb_ext.ap().rearrange("(a b) -> a b", b=1))
            gwf = cst.tile([P, 2 * M], F32)
            for kt in range(2):
                nc.sync.dma_start(gwf[:, kt * M:(kt + 1) * M],
                                  gw_ext[kt * P:(kt + 1) * P, :])

            eps_t = cst.tile([1, 1], F32)
            nc.any.memset(eps_t[:], LN_EPS)
            xT = cst.tile([P, 2 * BL], BF16)  # x^T k-tiles side by side [k, b]
            scT = cst.tile([M, BL], F32)      # local top-k scores^T

            # DRAM bounce buffers
            NCH = 8  # h1 AllGather chunks
            CHK = D // NCH  # 1024 features per chunk
            ag_h1_in = dram.tile([D, BL], BF16)
            ag_h1_out = [dram.tile([N_CORES * CHK, BL], BF16, addr_space="Shared",
                                   tag=f"ag_h1_out{j}", name=f"ag_h1_out{j}")
                         for j in range(NCH)]
            sc_in = dram.tile([M, BL], F32)
            sc_out = dram.tile([N_CORES * M, BL], F32, addr_space="Shared")
            st_in = [dram.tile([2, BL], F32, tag=f"st_in{r}", name=f"st_in{r}")
                     for r in range(NRB)]
            st_out = [dram.tile([2, BL], F32, addr_space="Shared",
                                tag=f"st_out{r}", name=f"st_out{r}")
                      for r in range(NRB)]
            hd_in = dram.tile([2 * ACT_DIM, B], F32)
            hd_out = dram.tile([2 * ACT_DIM, B], F32, addr_space="Shared")

            def ln_stats_to_bcast(pool, stx, stq, tagp, un):
                """[1,BL] f32 sum/sumsq vectors -> bf16 bcast tiles invB, nmuB."""
                def v(nm):
                    return pool.tile([1, BL], F32, tag=f"{tagp}v", bufs=6, name=f"{nm}{un}")
                mu = v("mu")
                nc.vector.tensor_scalar_mul(mu[:], stx[:], 1.0 / D)
                e2 = v("e2")
                nc.vector.tensor_scalar_mul(e2[:], stq[:], 1.0 / D)
                mu2 = v("mu2")
                nc.scalar.activation(mu2[:], mu[:], AF.Square)
                var = v("var")
                nc.vector.tensor_tensor(var[:], e2[:], mu2[:], op=ALU.subtract)
                sd = v("sd")
                nc.scalar.activation(sd[:], var[:], AF.Sqrt, bias=eps_t[:])
                inv = v("inv")
                nc.vector.reciprocal(inv[:], sd[:])
                nmuinv = v("nmuinv")
                nc.vector.tensor_tensor(nmuinv[:], mu[:], inv[:], op=ALU.mult)
                nmu2 = v("nmu2")
                nc.vector.tensor_scalar_mul(nmu2[:], nmuinv[:], -1.0)
                vb = pool.tile([1, 2 * BL], BF16, tag=f"{tagp}vb", bufs=2, name=f"vb{un}")
                nc.vector.tensor_copy(vb[:, 0:BL], inv[:])
                nc.vector.tensor_copy(vb[:, BL:2 * BL], nmu2[:])
                invB_ps = ppb.tile([P, BL], F32, tag="psb", name=f"invBps{un}")
                nc.tensor.matmul(invB_ps[:], ones_row_b[:], vb[:, 0:BL], start=True, stop=True)
                invB = pool.tile([P, BL], BF16, tag=f"{tagp}bc", bufs=3, name=f"invB{un}")
                nc.scalar.activation(invB[:], invB_ps[:], AF.Copy)
                nmuB_ps = ppb.tile([P, BL], F32, tag="psb", name=f"nmuBps{un}")
                nc.tensor.matmul(nmuB_ps[:], ones_row_b[:], vb[:, BL:2 * BL], start=True, stop=True)
                nmuB = pool.tile([P, BL], BF16, tag=f"{tagp}bc", bufs=3, name=f"nmuB{un}")
                nc.scalar.activation(nmuB[:], nmuB_ps[:], AF.Copy)
                return invB, nmuB

            # ================= phase 0 + 1 (pool p1) =======================
            with tc.tile_pool(name="p1", bufs=1) as p1:
                Smat_f = p1.tile([P, NNT * HL], F32)
                nc.sync.dma_start(Smat_f[:], S_dram[:])
                nc.vector.tensor_copy(Smat[:], Smat_f[:])
                # ---- x^T ----
                xTf = p1.tile([P, 2 * BL], F32, tag="xTf", bufs=1, name="xTf")
                for bt in range(NBT):
                    xl = p1.tile([P, OBS], F32, tag="xload", bufs=2, name=f"xl{bt}")
                    nc.sync.dma_start(xl[:], x_ext[bt * P:(bt + 1) * P, :])
                    for kt in range(2):
                        tp = ppb.tile([P, P], F32, tag="psb", name=f"xtp{bt}_{kt}")
                        nc.tensor.transpose(tp[:], xl[:, kt * P:(kt + 1) * P], ident[:])
                        nc.scalar.activation(
                            xTf[:, kt * BL + bt * P: kt * BL + (bt + 1) * P],
                            tp[:], AF.Copy)
                        nc.vector.tensor_copy(
                            xT[:, kt * BL + bt * P: kt * BL + (bt + 1) * P],
                            tp[:])

                # ---- gate + softmax + top-4 ----
                for bt in range(NBT):
                    gp = ppb.tile([P, M], F32, tag="psb", name=f"gp{bt}")
                    for kt in range(2):
                        nc.tensor.matmul(
                            gp[:], xTf[:, kt * BL + bt * P: kt * BL + (bt + 1) * P],
                            gwf[:, kt * M:(kt + 1) * M], start=(kt == 0), stop=False)
                    nc.tensor.matmul(gp[:], ones_row_f[:], gbf[:], start=False, stop=True)

                    def g1(nm):
                        return p1.tile([P, 1], F32, tag="gs1", bufs=6, name=f"{nm}{bt}")

                    def g16(nm):
                        return p1.tile([P, M], F32, tag="gs16", bufs=6, name=f"{nm}{bt}")

                    gmax = g1("gmax")
                    nc.vector.tensor_reduce(gmax[:], gp[:], AX.X, ALU.max)
                    ngmax = g1("ngmax")
                    nc.vector.tensor_scalar_mul(ngmax[:], gmax[:], -1.0)
                    ge = g16("ge")
                    nc.scalar.activation(ge[:], gp[:], AF.Exp, bias=ngmax[:])
                    gsum = g1("gsum")
                    nc.vector.reduce_sum(gsum[:], ge[:], axis=AX.X)
                    grec = g1("grec")
                    nc.vector.reciprocal(grec[:], gsum[:])
                    s0 = g16("s0")
                    nc.vector.tensor_scalar_mul(s0[:], ge[:], grec[:])
                    mt4 = p1.tile([P, TOPK], F32, tag="gs4", bufs=2, name=f"mt4{bt}")
                    w = s0
                    for t in range(TOPK):
                        nc.vector.tensor_reduce(mt4[:, t:t + 1], w[:], AX.X, ALU.max)
                        if t < TOPK - 1:
                            msk = g16(f"msk{t}_")
                            nc.vector.tensor_scalar(msk[:], w[:], mt4[:, t:t + 1], None,
                                                    op0=ALU.is_ge)
                            w2_ = g16(f"w{t}_")
                            nc.vector.tensor_tensor(w2_[:], w[:], msk[:], op=ALU.subtract)
                            w = w2_
                    tsum = g1("tsum")
                    nc.vector.reduce_sum(tsum[:], mt4[:], axis=AX.X)
                    trec = g1("trec")
                    nc.vector.reciprocal(trec[:], tsum[:])
                    keep = g16("keep")
                    nc.vector.tensor_scalar(keep[:], s0[:], mt4[:, TOPK - 1:TOPK], None,
                                            op0=ALU.is_ge)
                    sn = g16("sn")
                    nc.vector.tensor_scalar_mul(sn[:], s0[:], trec[:])
                    sc = g16("sc")
                    nc.vector.tensor_tensor(sc[:], sn[:], keep[:], op=ALU.mult)
                    stp = ppb.tile([M, P], F32, tag="psb", name=f"stp{bt}")
                    nc.tensor.transpose(stp[:], sc[:], ident[:])
                    nc.scalar.activation(scT[:, bt * P:(bt + 1) * P], stp[:], AF.Copy)
                nc.sync.dma_start(sc_in[:], scT[:])
                nc.gpsimd.collective_compute(
                    "AllGather", ALU.bypass, replica_groups=RG,
                    ins=[sc_in.opt()], outs=[sc_out.opt()])
                if DEBUG_TAPS:
                    nc.sync.dma_start(taps["scores"][:], scT[:])

                # ---- fc1 (bf16) + LN1 stats ----
                w1b = []
                for kt in range(2):
                    w1t = p1.tile([P, D], BF16, tag=f"w1b{kt}", bufs=1, name=f"w1b{kt}")
                    for h in range(2):
                        w1f = p1.tile([P, D // 2], F32, tag="w1f", bufs=2,
                                      name=f"w1f{kt}_{h}")
                        nc.sync.dma_start(
                            w1f[:], w1_ext[kt * P:(kt + 1) * P,
                                           h * (D // 2):(h + 1) * (D // 2)])
                        nc.vector.tensor_copy(w1t[:, h * (D // 2):(h + 1) * (D // 2)],
                                              w1f[:])
                    w1b.append(w1t)

                h1raw = []
                st1x = pps.tile([1, BL], F32, tag="pss", name="st1x")
                st1q = pps.tile([1, BL], F32, tag="pss", name="st1q")
                for nt in range(NKT):
                    ps1 = ppa.tile([P, BL], F32, tag="psa", name=f"ps1_{nt}")
                    for kt in range(2):
                        nc.tensor.matmul(ps1[:], w1b[kt][:, nt * P:(nt + 1) * P],
                                         xT[:, kt * BL:(kt + 1) * BL],
                                         start=(kt == 0), stop=(kt == 1))
                    h1r = p1.tile([P, BL], BF16, tag=f"h1raw{nt}", bufs=1,
                                  name=f"h1r{nt}")
                    nc.scalar.activation(h1r[:], ps1[:], AF.Identity,
                                         bias=fc1b[:, nt:nt + 1])
                    h1raw.append(h1r)
                    sq = p1.tile([P, BL], BF16, tag="sq1", bufs=3, name=f"sq1_{nt}")
                    nc.scalar.activation(sq[:], h1r[:], AF.Square)
                    nc.tensor.matmul(st1x[:], ones_col_b[:], h1r[:],
                                     start=(nt == 0), stop=(nt == NKT - 1))
                    nc.tensor.matmul(st1q[:], ones_col_b[:], sq[:],
                                     start=(nt == 0), stop=(nt == NKT - 1))

                sx1 = p1.tile([1, BL], F32, tag="ln1v", bufs=6, name="sx1")
                nc.vector.tensor_copy(sx1[:], st1x[:])
                sq1v = p1.tile([1, BL], F32, tag="ln1v", bufs=6, name="sq1v")
                nc.vector.tensor_copy(sq1v[:], st1q[:])
                invB1, nmuB1 = ln_stats_to_bcast(p1, sx1, sq1v, "ln1", "L1")

                # ---- LN1 normalize + relu -> AG input ----
                for nt in range(NKT):
                    u = p1.tile([P, BL], BF16, tag="n1u", bufs=3, name=f"u{nt}")
                    nc.vector.tensor_tensor(u[:], h1raw[nt][:], invB1[:], op=ALU.mult)
                    v_ = p1.tile([P, BL], BF16, tag="n1v", bufs=3, name=f"v{nt}")
                    nc.vector.tensor_tensor(v_[:], u[:], nmuB1[:], op=ALU.add)
                    hn = p1.tile([P, BL], BF16, tag="n1h", bufs=3, name=f"hn{nt}")
                    nc.scalar.activation(hn[:], v_[:], AF.Relu,
                                         scale=n1s[:, nt:nt + 1], bias=n1b[:, nt:nt + 1])
                    nc.sync.dma_start(ag_h1_in[nt * P:(nt + 1) * P, :], hn[:])
                    if DEBUG_TAPS and nt == 3:
                        hf = p1.tile([P, BL], F32, tag="tapf", bufs=1, name="hf")
                        nc.vector.tensor_copy(hf[:], hn[:])
                        nc.sync.dma_start(taps["h1"][:], hf[:])
                    if (nt + 1) % (NKT // NCH) == 0:
                        j = nt // (NKT // NCH)
                        nc.gpsimd.collective_compute(
                            "AllGather", ALU.bypass, replica_groups=RG,
                            ins=[ag_h1_in[j * CHK:(j + 1) * CHK, :].opt()],
                            outs=[ag_h1_out[j].opt()])

            # ================= phase 2: fc2 + LN2 + mixture ================
            with (
                tc.tile_pool(name="p2w", bufs=1) as p2w,
                tc.tile_pool(name="p2b", bufs=1) as p2b,
            ):
                w2sb = [None] * NKT

                def stage_A(r):
                    ps2 = [None] * NNT
                    for half in range(2):
                        nlo = half * (NNT // 2)
                        for n in range(nlo, nlo + NNT // 2):
                            ps2[n] = ppa.tile([P, BL], F32, tag="psa",
                                              name=f"ps2_{r}_{n}")
                        for k in range(NKT):
                            if r == 0 and half == 0:
                                w2t = p2w.tile([P, DL], BF16, tag=f"w2b{k}", bufs=1,
                                               name=f"w2b{k}")
                                for h in range(2):
                                    w2f = p2w.tile([P, DL // 2], F32, tag="w2f", bufs=3,
                                                   name=f"w2f{k}_{h}")
                                    nc.sync.dma_start(
                                        w2f[:], w2_ext[k * P:(k + 1) * P,
                                                       h * (DL // 2):(h + 1) * (DL // 2)])
                                    nc.vector.tensor_copy(
                                        w2t[:, h * (DL // 2):(h + 1) * (DL // 2)], w2f[:])
                                w2sb[k] = w2t
                            h1t = p2w.tile([P, BL], BF16, tag="h1s", bufs=4,
                                           name=f"h1t{r}_{half}_{k}")
                            j, kk = divmod(k, NKT // NCH)
                            nc.sync.dma_start(
                                h1t[:],
                                ag_h1_out[j][r * CHK + kk * P: r * CHK + (kk + 1) * P, :])
                            for n in range(nlo, nlo + NNT // 2):
                                nc.tensor.matmul(ps2[n][:],
                                                 w2sb[k][:, n * P:(n + 1) * P], h1t[:],
                                                 start=(k == 0), stop=(k == NKT - 1))
                    h2r = p2b.tile([P, NNT * BL], BF16, tag="h2raw", bufs=2,
                                   name=f"h2r{r}")
                    st2x = pps.tile([1, BL], F32, tag="pss", name=f"st2x{r}")
                    st2q = pps.tile([1, BL], F32, tag="pss", name=f"st2q{r}")
                    for n in range(NNT):
                        nc.scalar.activation(h2r[:, n * BL:(n + 1) * BL], ps2[n][:],
                                             AF.Identity, bias=fc2b[:, n:n + 1])
                        sq = p2b.tile([P, BL], BF16, tag="sq2", bufs=2,
                                      name=f"sq2_{r}_{n}")
                        nc.scalar.activation(sq[:], h2r[:, n * BL:(n + 1) * BL], AF.Square)
                        nc.tensor.matmul(st2x[:], ones_col_b[:],
                                         h2r[:, n * BL:(n + 1) * BL],
                                         start=(n == 0), stop=(n == NNT - 1))
                        nc.tensor.matmul(st2q[:], ones_col_b[:], sq[:],
                                         start=(n == 0), stop=(n == NNT - 1))
                    sx = p2b.tile([1, BL], F32, tag="ln2v", bufs=6, name=f"sx{r}")
                    nc.vector.tensor_copy(sx[:], st2x[:])
                    sq_ = p2b.tile([1, BL], F32, tag="ln2v", bufs=6, name=f"sqe{r}")
                    nc.vector.tensor_copy(sq_[:], st2q[:])
                    nc.sync.dma_start(st_in[r][0:1, :], sx[:])
                    nc.sync.dma_start(st_in[r][1:2, :], sq_[:])
                    nc.gpsimd.collective_compute(
                        "AllReduce", ALU.add, replica_groups=RG,
                        ins=[st_in[r].opt()], outs=[st_out[r].opt()])
                    return h2r

                def stage_B(r, h2r):
                    stx = p2b.tile([1, BL], F32, tag="ln2v", bufs=6, name=f"stxb{r}")
                    nc.sync.dma_start(stx[:], st_out[r][0:1, :])
                    stq = p2b.tile([1, BL], F32, tag="ln2v", bufs=6, name=f"stqb{r}")
                    nc.sync.dma_start(stq[:], st_out[r][1:2, :])
                    invB, nmuB = ln_stats_to_bcast(p2b, stx, stq, "ln2", f"B{r}")
                    scr = p2b.tile([M, BL], F32, tag="scr", bufs=1, name=f"scr{r}")
                    nc.sync.dma_start(scr[:], sc_out[r * M:(r + 1) * M, :])
                    scrb = p2b.tile([M, BL], BF16, tag="scrb", bufs=2, name=f"scrb{r}")
                    nc.vector.tensor_copy(scrb[:], scr[:])
                    scB_ps = ppb.tile([P, BL], F32, tag="psb", name=f"scBps{r}")
                    nc.tensor.matmul(scB_ps[:], Emat[:], scrb[:], start=True, stop=True)
                    scB = p2b.tile([P, BL], BF16, tag="scB", bufs=2, name=f"scB{r}")
                    nc.scalar.activation(scB[:], scB_ps[:], AF.Copy)
                    mix_ps = ppb.tile([HL, BL], F32, tag="psb", name=f"mixps{r}")
                    for n in range(NNT):
                        u = p2b.tile([P, BL], BF16, tag="n2u", bufs=2, name=f"u2_{r}_{n}")
                        nc.vector.tensor_tensor(u[:], h2r[:, n * BL:(n + 1) * BL],
                                                invB[:], op=ALU.mult)
                        v_ = p2b.tile([P, BL], BF16, tag="n2v", bufs=2, name=f"v2_{r}_{n}")
                        nc.vector.tensor_tensor(v_[:], u[:], nmuB[:], op=ALU.add)
                        w_ = p2b.tile([P, BL], BF16, tag="n2w", bufs=2, name=f"w2_{r}_{n}")
                        nc.scalar.activation(w_[:], v_[:], AF.Relu,
                                             scale=n2s[:, n:n + 1], bias=n2b[:, n:n + 1])
                        pr = p2b.tile([P, BL], BF16, tag="n2p", bufs=2, name=f"p2_{r}_{n}")
                        nc.vector.tensor_tensor(pr[:], w_[:], scB[:], op=ALU.mult)
                        nc.tensor.matmul(mix_ps[:], Smat[:, n * HL:(n + 1) * HL], pr[:],
                                         start=(n == 0), stop=(n == NNT - 1))
                    mx = p2b.tile([HL, BL], BF16, tag="mx", bufs=2, name=f"mx{r}")
                    nc.scalar.activation(mx[:], mix_ps[:], AF.Copy)
                    if DEBUG_TAPS and r == 0:
                        mxf = p2b.tile([HL, BL], F32, tag="mxf", bufs=1, name=f"mxf{r}")
                        nc.vector.tensor_copy(mxf[:], mx[:])
                        nc.sync.dma_start(taps["mixed"][:], mxf[:])
                    hp = ppb.tile([2 * ACT_DIM, BL], F32, tag="psb", name=f"hp{r}")
                    nc.tensor.matmul(hp[:], hwt[:], mx[:], start=True, stop=True)
                    hpe = p2b.tile([2 * ACT_DIM, BL], F32, tag="hpe", bufs=2, name=f"hpe{r}")
                    nc.vector.tensor_copy(hpe[:], hp[:])
                    nc.sync.dma_start(hd_in[0:ACT_DIM, r * BL:(r + 1) * BL],
                                      hpe[0:ACT_DIM, :])
                    nc.sync.dma_start(hd_in[ACT_DIM:2 * ACT_DIM, r * BL:(r + 1) * BL],
                                      hpe[ACT_DIM:2 * ACT_DIM, :])

                h2_prev = stage_A(0)
                for r in range(1, NRB):
                    h2_cur = stage_A(r)
                    stage_B(r - 1, h2_prev)
                    h2_prev = h2_cur
                stage_B(NRB - 1, h2_prev)

                nc.gpsimd.collective_compute(
                    "AllReduce", ALU.add, replica_groups=RG,
                    ins=[hd_in.opt()], outs=[hd_out.opt()])

            # ================= phase 3: epilogue ===========================
            with tc.tile_pool(name="p3", bufs=1) as p3:
                for bb in range(NRB):
                    hm = p3.tile([ACT_DIM, BL], F32, tag="hm", bufs=2, name=f"hm{bb}")
                    nc.sync.dma_start(hm[:], hd_out[0:ACT_DIM, bb * BL:(bb + 1) * BL])
                    mfin = p3.tile([ACT_DIM, BL], F32, tag="mf", bufs=2, name=f"mf{bb}")
                    nc.scalar.activation(mfin[:], hm[:], AF.Identity, bias=mbt[:])
                    nc.sync.dma_start(out_ext[0:ACT_DIM, bb * BL:(bb + 1) * BL], mfin[:])
                    hl = p3.tile([ACT_DIM, BL], F32, tag="hlr", bufs=2, name=f"hl{bb}")
                    nc.sync.dma_start(hl[:], hd_out[ACT_DIM:2 * ACT_DIM,
                                                    bb * BL:(bb + 1) * BL])
                    th = p3.tile([ACT_DIM, BL], F32, tag="th", bufs=2, name=f"th{bb}")
                    nc.scalar.activation(th[:], hl[:], AF.Tanh, bias=lbt[:])
                    lfin = p3.tile([ACT_DIM, BL], F32, tag="lf", bufs=2, name=f"lf{bb}")
                    nc.vector.tensor_scalar(
                        lfin[:], th[:], 0.5 * (LOG_STD_MAX - LOG_STD_MIN),
                        LOG_STD_MIN + 0.5 * (LOG_STD_MAX - LOG_STD_MIN),
                        op0=ALU.mult, op1=ALU.add)
                    nc.sync.dma_start(out_ext[ACT_DIM:2 * ACT_DIM,
                                              bb * BL:(bb + 1) * BL], lfin[:])

    nc.compile()
    return nc


_NC_CACHE = None


def _get_nc():
    global _NC_CACHE
    if _NC_CACHE is None:
        _NC_CACHE = build_kernel()
    return _NC_CACHE


def make_in_maps(inputs):
    def f32c(a):
        return np.ascontiguousarray(np.asarray(a, np.float32))

    x = f32c(inputs["x"])
    fc2_W = f32c(inputs["fc2_W"])
    in_maps = []
    for i in range(N_CORES):
        in_maps.append({
            "x": np.ascontiguousarray(x[i * BL:(i + 1) * BL]),
            "gate_W": f32c(inputs["gate_W"]),
            "gate_b": f32c(inputs["gate_b"]),
            "fc1_W": f32c(inputs["fc1_W"]),
            "fc1_b": f32c(inputs["fc1_b"]),
            "norm1_scale": f32c(inputs["norm1_scale"]),
            "norm1_bias": f32c(inputs["norm1_bias"]),
            "fc2_W": np.ascontiguousarray(fc2_W[:, i * DL:(i + 1) * DL]),
            "fc2_b": f32c(inputs["fc2_b"])[i * DL:(i + 1) * DL].copy(),
            "norm2_scale": f32c(inputs["norm2_scale"])[i * DL:(i + 1) * DL].copy(),
            "norm2_bias": f32c(inputs["norm2_bias"])[i * DL:(i + 1) * DL].copy(),
            "mean_W": f32c(inputs["mean_W"])[i * HL:(i + 1) * HL, :].copy(),
            "mean_b": f32c(inputs["mean_b"]),
            "logstd_W": f32c(inputs["logstd_W"])[i * HL:(i + 1) * HL, :].copy(),
            "logstd_b": f32c(inputs["logstd_b"]),
        })
    return in_maps


def kernel(**inputs):
    topk = int(inputs.get("topk", TOPK))
    assert topk == TOPK, f"kernel compiled for topk={TOPK}, got {topk}"
    nc = _get_nc()
    in_maps = make_in_maps(inputs)
    res = run_bass_kernel_spmd(nc, in_maps, core_ids=list(range(N_CORES)))
    out = res.results[0]["out"]  # [64, B]; identical on every core
    mean = np.ascontiguousarray(out[:ACT_DIM, :].T)
    log_std = np.ascontiguousarray(out[ACT_DIM:, :].T)
    return mean, log_std
